# revision 13
# baseline (speedup 1.0000x reference)
"""Correlation (FlowNet-style, max_displacement=4) on 8 TRN2 NeuronCores.

Full inputs x1, x2: [B=8, C=64, H=192, W=192] fp32. Output: [8, 81, 192, 192] fp32.
out[b, di*9+dj, h, w] = mean_c x1[b,c,h,w] * x2pad[b,c,h+di,w+dj]   (di,dj in [0,9))

Strategy: batch-parallel (1 batch per core). Per core the correlation is a banded
Gram matrix on the TensorEngine: for each 16x8 (h,w) output tile, one bf16 matmul
with lhsT = x1 tile [K=64 channels, M=128 pixels] and rhs = padded x2 window
[64, 24*16=384 pixels] produces all 81 displacement dot products of every tile
pixel inside a skewed band of the 128x384 PSUM result. 16x8 tiles (vs 8x16) keep
the same matmul shape but shrink the band rectangles: per dh-group g (8
partitions) the useful columns are [16g, 16g+144), so adjacent group-pairs form
8 rectangles per strip at only 1.11x the useful bytes.

Two matmuls (2 w-tiles, one h-half) share a 2-bank PSUM tile (bufs=4 keeps the
MM->evict pipeline full; 4-bank quads would cap PSUM at 2 tiles and serialize
MMs behind evictions). PSUM is evicted (fp32->bf16) by DVE/ACT alternately
(both run 1x mode on fp32 PSUM reads - the 32b/lane/cycle PSUM port is the
eviction floor) into a per-strip ybuf [128, 384, 2, 24] covering both h-halves.
Output DMA is 8 big rect ops per strip ([16 part x 15.4 KB run] = 246 KB each),
keeping the HWDGE descriptor generator (~650ns/op) off the critical path; the
last strip uses per-half buffers so its drain overlaps compute. Band deskew
happens on the host with a zero-copy strided view. x1 is pre-scaled by 1/64.

The h axis is split into two halves on partitions 0-63 / 64-127 so paired
matmuls (K=64 each) run concurrently on disjoint PE row-groups. Inputs load in
three h-chunks, issued lazily just before the first strip that needs them, so
chunk 0 is not starved of SDMA bandwidth at startup.
"""

import sys
import types

import numpy as np
import ml_dtypes

import concourse.bacc as bacc
from concourse import mybir
from concourse.tile import TileContext
from concourse.bass_utils import run_bass_kernel_spmd

B, C, H, W = 8, 64, 192, 192
MAXD = 4
D = 2 * MAXD + 1  # 9
HP, WP = H + 2 * MAXD, W + 2 * MAXD  # 200, 200

TH, TW = 16, 8            # output tile (h, w) -> M = 128
NH, NW = TH + 2 * MAXD, TW + 2 * MAXD  # x2 window 24 x 16 -> N = 384
NSP = H // (2 * TH)       # 6 strips per partition-half
N_WT = W // TW            # 24 w-tiles
HHALF = H // 2            # 96 rows per partition-half
SLAB = HHALF + 2 * MAXD   # 104 padded x2 rows per half
RC = 160                  # columns per rect (2 dh-groups)
RP = 16                   # partitions per rect

# Input h-chunking: strip ranges per chunk and the x2 slab rows they need.
X1_CHUNKS = [(0, 1), (1, 3), (3, 6)]
X2_CHUNKS = [(0, 24), (16, 56), (48, 104)]

BF16 = ml_dtypes.bfloat16


def _install_axon_trace_shim():
    """The image's antenv package lacks axon_hooks; run_bass_kernel_spmd
    crashes on import when trace=True. Provide the hook from the boot module
    so tracing works instead of raising."""
    if "antenv.axon_hooks" in sys.modules:
        return
    try:
        import trn_agent_boot.trn_boot as tb

        hook = tb._ntff_profile_via_ctypes("/opt/axon/libaxon_pjrt.so")
    except Exception:
        hook = None
    mod = types.ModuleType("antenv.axon_hooks")
    mod.get_axon_ntff_profile_hook = lambda: hook
    mod.set_axon_ntff_profile_hook = lambda h: None
    sys.modules["antenv.axon_hooks"] = mod


def build_nc():
    nc = bacc.Bacc("TRN2", target_bir_lowering=False, debug=False)
    # x1 arrives pre-tiled: [128 = half*64+c, strip, wtile, 128 pixels] - walrus
    # requires the matmul weights AP to have a single free dimension.
    x1s = nc.dram_tensor("x1s", [128, NSP, N_WT, TH * TW], mybir.dt.bfloat16, kind="ExternalInput")
    x2s = nc.dram_tensor("x2s", [128, SLAB, WP], mybir.dt.bfloat16, kind="ExternalInput")
    y = nc.dram_tensor("y", [NSP - 1, 8, RP, RC, 2, N_WT], mybir.dt.bfloat16, kind="ExternalOutput")
    # The last strip uses per-half buffers so its first half's output DMA
    # overlaps the second half's eviction, shortening the drain tail.
    yl = nc.dram_tensor("yl", [2, 8, RP, RC, N_WT], mybir.dt.bfloat16, kind="ExternalOutput")

    with TileContext(nc) as tc:
        with (
            tc.tile_pool(name="imgs", bufs=1) as imgs,
            tc.tile_pool(name="outs", bufs=2) as outs,
            tc.tile_pool(name="psum", bufs=4, space="PSUM") as psum,
        ):
            # Chunked input tiles (separate tiles -> precise chunk->matmul
            # deps). Loads are issued lazily, right before the first strip
            # that needs them, so chunk 0 isn't starved of SDMA bandwidth by
            # later chunks at startup.
            x1c, x2c = [], []

            def load_chunk(ci):
                s0, s1 = X1_CHUNKS[ci]
                r0, r1 = X2_CHUNKS[ci]
                x2t = imgs.tile([128, r1 - r0, WP], mybir.dt.bfloat16, tag=f"x2c{ci}")
                nc.sync.dma_start(out=x2t[:], in_=x2s[:, r0:r1, :])
                x1t = imgs.tile([128, s1 - s0, N_WT, TH * TW], mybir.dt.bfloat16, tag=f"x1c{ci}")
                nc.sync.dma_start(out=x1t[:], in_=x1s[:, s0:s1])
                x2c.append(x2t)
                x1c.append(x1t)

            load_chunk(0)

            copy_k = 0
            for sp in range(NSP):
                ci = next(i for i, (s0, s1) in enumerate(X1_CHUNKS) if s0 <= sp < s1)
                if ci + 1 < len(X1_CHUNKS) and len(x1c) == ci + 1:
                    load_chunk(ci + 1)
                hl = sp * TH - X2_CHUNKS[ci][0]   # row offset within x2 chunk
                spl = sp - X1_CHUNKS[ci][0]       # strip offset within x1 chunk
                last = sp == NSP - 1

                def mm_pair(half, tp, pt):
                    p0 = 64 * half
                    for u in range(2):
                        t = 2 * tp + u
                        w0 = t * TW
                        nc.tensor.matmul(
                            pt[:, 512 * u:512 * u + NH * NW],
                            lhsT=x1c[ci][p0:p0 + 64, spl, t, :],
                            rhs=x2c[ci][p0:p0 + 64, hl:hl + NH, w0:w0 + NW],
                            start=True, stop=True,
                        )

                def evict(pt, dst):
                    nonlocal copy_k
                    src = pt[:].rearrange("p (u b) -> p b u", u=2)[:, 0:NH * NW]
                    if copy_k % 2 == 0:
                        nc.vector.tensor_copy(dst, src)
                    else:
                        nc.scalar.copy(dst, src)
                    copy_k += 1

                # Band rects out: for rect r, columns [32r, 32r+160) of
                # partitions [16r, 16r+16) hold all (di, dj) results for those
                # rows - one contiguous run per partition.
                if not last:
                    # One ybuf per strip covering both h-halves; [col, half,
                    # wtile] layout makes each rect one contiguous run per
                    # partition. Interleave the two partition halves so
                    # adjacent matmuls sit on disjoint PE row-groups and
                    # execute concurrently.
                    ybuf = outs.tile([128, NH * NW, 2, N_WT], mybir.dt.bfloat16,
                                     name=f"ybuf{sp}", tag="ybuf")
                    for tp in range(N_WT // 2):   # pairs of w-tiles
                        for half in range(2):
                            pt = psum.tile([128, 1024], mybir.dt.float32)
                            mm_pair(half, tp, pt)
                            evict(pt, ybuf[:, :, half, 2 * tp:2 * tp + 2])
                    for r in range(8):
                        nc.sync.dma_start(
                            out=y[sp, r],
                            in_=ybuf[RP * r:RP * r + RP, 32 * r:32 * r + RC],
                        )
                else:
                    # Last strip: per-half buffers, half 0's rects drain while
                    # half 1 computes, halving the end-of-kernel tail.
                    for half in range(2):
                        ybh = outs.tile([128, NH * NW, N_WT], mybir.dt.bfloat16,
                                        name=f"ybl{half}", tag="ybl")
                        for tp in range(N_WT // 2):
                            pt = psum.tile([128, 1024], mybir.dt.float32)
                            mm_pair(half, tp, pt)
                            evict(pt, ybh[:, :, 2 * tp:2 * tp + 2])
                        for r in range(8):
                            nc.sync.dma_start(
                                out=yl[half, r],
                                in_=ybh[RP * r:RP * r + RP, 32 * r:32 * r + RC],
                            )

    nc.compile()
    return nc


_NC_CACHE = None


def _get_nc():
    global _NC_CACHE
    if _NC_CACHE is None:
        _NC_CACHE = build_nc()
    return _NC_CACHE


def _prep_inputs(x1, x2):
    """Host-side shard prep: scale, pad, split h into partition halves, bf16."""
    in_maps = []
    x1 = np.asarray(x1, dtype=np.float32)
    x2 = np.asarray(x2, dtype=np.float32)
    x1h = (x1 * (1.0 / C)).astype(BF16)
    x2h = x2.astype(BF16)
    for b in range(B):
        # x1: [64, 192, 192] -> pre-tiled [128 = half*64+c, sp, t, dh*8+dw]
        a = x1h[b].reshape(C, 2, NSP, TH, N_WT, TW)
        a = a.transpose(1, 0, 2, 4, 3, 5).reshape(128, NSP, N_WT, TH * TW)
        # x2: pad to [64, 200, 200], two overlapping 104-row slabs
        p = np.zeros((C, HP, WP), dtype=BF16)
        p[:, MAXD:MAXD + H, MAXD:MAXD + W] = x2h[b]
        s = np.stack([p[:, 0:SLAB, :], p[:, HHALF:HHALF + SLAB, :]], axis=0)
        s = s.reshape(2 * C, SLAB, WP)
        in_maps.append({"x1s": np.ascontiguousarray(a), "x2s": np.ascontiguousarray(s)})
    return in_maps


def _deskew(yb, ylb):
    """yb: [5, 8, 16, 160, 2, 24], ylb: [2, 8, 16, 160, 24] fp32 (one batch)
    -> [81, 192, 192] fp32.

    y[sp, r, p'', c'', half, t] where pixel (dh=2r+r2, dw), p'' = 8*r2+dw, and
    c'' = 16*(r2+di) + dw + dj for displacement (di, dj). yl is the last strip
    with half as the leading axis.
    """
    out = np.empty((D * D, 2, NSP, TH, N_WT, TW), dtype=np.float32)
    s_sp, s_r, s_p, s_c, s_half, s_t = yb.strides
    v = np.lib.stride_tricks.as_strided(
        yb,
        shape=(D, D, 2, NSP - 1, 8, 2, N_WT, TW),
        strides=(16 * s_c, s_c, s_half, s_sp, s_r,
                 8 * s_p + 16 * s_c, s_t, s_p + s_c),
    )
    # axes: di, dj, half, sp, r, r2, t, dw
    out[:, :, :NSP - 1] = v.reshape(D * D, 2, NSP - 1, TH, N_WT, TW)
    s_half, s_r, s_p, s_c, s_t = ylb.strides
    vl = np.lib.stride_tricks.as_strided(
        ylb,
        shape=(D, D, 2, 8, 2, N_WT, TW),
        strides=(16 * s_c, s_c, s_half, s_r, 8 * s_p + 16 * s_c, s_t, s_p + s_c),
    )
    out[:, :, NSP - 1] = vl.reshape(D * D, 2, TH, N_WT, TW)
    # (half, sp, dh) -> h, (t, dw) -> w
    return out.transpose(0, 1, 2, 3, 4, 5).reshape(D * D, H, W)


def kernel(x1, x2):
    _install_axon_trace_shim()
    nc = _get_nc()
    in_maps = _prep_inputs(x1, x2)
    res = run_bass_kernel_spmd(nc, in_maps, core_ids=list(range(B)))
    kernel.last_results = res
    out = np.empty((B, D * D, H, W), dtype=np.float32)
    for b in range(B):
        yb = np.asarray(res.results[b]["y"]).astype(np.float32)
        ylb = np.asarray(res.results[b]["yl"]).astype(np.float32)
        out[b] = _deskew(yb, ylb)
    return out


# revision 17
# speedup vs baseline: 1.0126x; 1.0126x over previous
"""Correlation (FlowNet-style, max_displacement=4) on 8 TRN2 NeuronCores.

Full inputs x1, x2: [B=8, C=64, H=192, W=192] fp32. Output: [8, 81, 192, 192] fp32.
out[b, di*9+dj, h, w] = mean_c x1[b,c,h,w] * x2pad[b,c,h+di,w+dj]   (di,dj in [0,9))

Strategy: batch-parallel (1 batch per core). Per core the correlation is a banded
Gram matrix on the TensorEngine: for each 16x8 (h,w) output tile, one bf16 matmul
with lhsT = x1 tile [K=64 channels, M=128 pixels] and rhs = padded x2 window
[64, 24*16=384 pixels] produces all 81 displacement dot products of every tile
pixel inside a skewed band of the 128x384 PSUM result. 16x8 tiles (vs 8x16) keep
the same matmul shape but shrink the band rectangles: per dh-group g (8
partitions) the useful columns are [16g, 16g+144), so adjacent group-pairs form
8 rectangles per strip at only 1.11x the useful bytes.

Two matmuls (2 w-tiles, one h-half) share a 2-bank PSUM tile (bufs=4 keeps the
MM->evict pipeline full; 4-bank quads would cap PSUM at 2 tiles and serialize
MMs behind evictions). PSUM is evicted (fp32->bf16) by DVE/ACT alternately into
a per-strip ybuf [128, 384, 2, 24] covering both h-halves. Output DMA is 8 big
rect ops per strip ([16 part x 15.4 KB run] = 246 KB each, 48 total), keeping
the HWDGE descriptor generator (~650ns/op) off the critical path. Band deskew
happens on the host with a zero-copy strided view. x1 is pre-scaled by 1/64.

The h axis is split into two halves on partitions 0-63 / 64-127 so paired
matmuls (K=64 each) run concurrently on disjoint PE row-groups. Inputs load in
four h-chunks interleaved with compute so the PE starts early.
"""

import sys
import types

import numpy as np
import ml_dtypes

import concourse.bacc as bacc
from concourse import mybir
from concourse.tile import TileContext
from concourse.bass_utils import run_bass_kernel_spmd

B, C, H, W = 8, 64, 192, 192
MAXD = 4
D = 2 * MAXD + 1  # 9
HP, WP = H + 2 * MAXD, W + 2 * MAXD  # 200, 200

TH, TW = 16, 8            # output tile (h, w) -> M = 128
NH, NW = TH + 2 * MAXD, TW + 2 * MAXD  # x2 window 24 x 16 -> N = 384
NSP = H // (2 * TH)       # 6 strips per partition-half
N_WT = W // TW            # 24 w-tiles
HHALF = H // 2            # 96 rows per partition-half
SLAB = HHALF + 2 * MAXD   # 104 padded x2 rows per half
RC = 160                  # columns per rect (2 dh-groups)
RP = 16                   # partitions per rect

# Input h-chunking: strip ranges per chunk and the x2 slab rows they need.
X1_CHUNKS = [(0, 1), (1, 3), (3, 6)]
X2_CHUNKS = [(0, 24), (16, 56), (48, 104)]

BF16 = ml_dtypes.bfloat16


def _install_axon_trace_shim():
    """The image's antenv package lacks axon_hooks; run_bass_kernel_spmd
    crashes on import when trace=True. Provide the hook from the boot module
    so tracing works instead of raising."""
    if "antenv.axon_hooks" in sys.modules:
        return
    try:
        import trn_agent_boot.trn_boot as tb

        hook = tb._ntff_profile_via_ctypes("/opt/axon/libaxon_pjrt.so")
    except Exception:
        hook = None
    mod = types.ModuleType("antenv.axon_hooks")
    mod.get_axon_ntff_profile_hook = lambda: hook
    mod.set_axon_ntff_profile_hook = lambda h: None
    sys.modules["antenv.axon_hooks"] = mod


def build_nc():
    nc = bacc.Bacc("TRN2", target_bir_lowering=False, debug=False)
    # x1 arrives pre-tiled: [128 = half*64+c, strip, wtile, 128 pixels] - walrus
    # requires the matmul weights AP to have a single free dimension.
    x1s = nc.dram_tensor("x1s", [128, NSP, N_WT, TH * TW], mybir.dt.bfloat16, kind="ExternalInput")
    x2s = nc.dram_tensor("x2s", [128, SLAB, WP], mybir.dt.bfloat16, kind="ExternalInput")
    y = nc.dram_tensor("y", [NSP, 8, RP, RC, 2, N_WT], mybir.dt.bfloat16, kind="ExternalOutput")

    with TileContext(nc) as tc:
        with (
            tc.tile_pool(name="imgs", bufs=1) as imgs,
            tc.tile_pool(name="outs", bufs=3) as outs,
            tc.tile_pool(name="psum", bufs=4, space="PSUM") as psum,
        ):
            # Chunked input tiles (separate tiles -> precise chunk->matmul
            # deps). Loads are issued lazily, right before the first strip
            # that needs them, so chunk 0 isn't starved of SDMA bandwidth by
            # later chunks at startup.
            x1c, x2c = [], []

            def load_chunk(ci):
                s0, s1 = X1_CHUNKS[ci]
                r0, r1 = X2_CHUNKS[ci]
                x2t = imgs.tile([128, r1 - r0, WP], mybir.dt.bfloat16, tag=f"x2c{ci}")
                nc.sync.dma_start(out=x2t[:], in_=x2s[:, r0:r1, :])
                x1t = imgs.tile([128, s1 - s0, N_WT, TH * TW], mybir.dt.bfloat16, tag=f"x1c{ci}")
                nc.sync.dma_start(out=x1t[:], in_=x1s[:, s0:s1])
                x2c.append(x2t)
                x1c.append(x1t)

            load_chunk(0)

            copy_k = 0
            for sp in range(NSP):
                ci = next(i for i, (s0, s1) in enumerate(X1_CHUNKS) if s0 <= sp < s1)
                if ci + 1 < len(X1_CHUNKS) and len(x1c) == ci + 1:
                    load_chunk(ci + 1)
                hl = sp * TH - X2_CHUNKS[ci][0]   # row offset within x2 chunk
                spl = sp - X1_CHUNKS[ci][0]       # strip offset within x1 chunk
                # One ybuf per strip covering both h-halves; [col, half, wtile]
                # layout makes each rect one contiguous run per partition.
                ybuf = outs.tile([128, NH * NW, 2, N_WT], mybir.dt.bfloat16,
                                 name=f"ybuf{sp}", tag="ybuf")
                for tp in range(N_WT // 2):       # pairs of w-tiles
                    # Interleave the two partition halves so adjacent matmuls
                    # sit on disjoint PE row-groups and execute concurrently.
                    for half in range(2):
                        p0 = 64 * half
                        pt = psum.tile([128, 1024], mybir.dt.float32)
                        for u in range(2):
                            t = 2 * tp + u
                            w0 = t * TW
                            nc.tensor.matmul(
                                pt[:, 512 * u:512 * u + NH * NW],
                                lhsT=x1c[ci][p0:p0 + 64, spl, t, :],
                                rhs=x2c[ci][p0:p0 + 64, hl:hl + NH, w0:w0 + NW],
                                start=True, stop=True,
                            )
                        # Evict both tiles with one op; alternate DVE / ACT.
                        src = pt[:].rearrange("p (u b) -> p b u", u=2)[:, 0:NH * NW]
                        dst = ybuf[:, :, half, 2 * tp:2 * tp + 2]
                        if copy_k % 2 == 0:
                            nc.vector.tensor_copy(dst, src)
                        else:
                            nc.scalar.copy(dst, src)
                        copy_k += 1
                # Band rects out: for rect r, columns [32r, 32r+160) of
                # partitions [16r, 16r+16) hold all (di, dj) results for those
                # rows - one contiguous 15.4 KB run per partition, 246 KB per op.
                for r in range(8):
                    nc.sync.dma_start(
                        out=y[sp, r],
                        in_=ybuf[RP * r:RP * r + RP, 32 * r:32 * r + RC],
                    )

    nc.compile()
    return nc


_NC_CACHE = None


def _get_nc():
    global _NC_CACHE
    if _NC_CACHE is None:
        _NC_CACHE = build_nc()
    return _NC_CACHE


def _prep_inputs(x1, x2):
    """Host-side shard prep: scale, pad, split h into partition halves, bf16."""
    in_maps = []
    x1 = np.asarray(x1, dtype=np.float32)
    x2 = np.asarray(x2, dtype=np.float32)
    x1h = (x1 * (1.0 / C)).astype(BF16)
    x2h = x2.astype(BF16)
    for b in range(B):
        # x1: [64, 192, 192] -> pre-tiled [128 = half*64+c, sp, t, dh*8+dw]
        a = x1h[b].reshape(C, 2, NSP, TH, N_WT, TW)
        a = a.transpose(1, 0, 2, 4, 3, 5).reshape(128, NSP, N_WT, TH * TW)
        # x2: pad to [64, 200, 200], two overlapping 104-row slabs
        p = np.zeros((C, HP, WP), dtype=BF16)
        p[:, MAXD:MAXD + H, MAXD:MAXD + W] = x2h[b]
        s = np.stack([p[:, 0:SLAB, :], p[:, HHALF:HHALF + SLAB, :]], axis=0)
        s = s.reshape(2 * C, SLAB, WP)
        in_maps.append({"x1s": np.ascontiguousarray(a), "x2s": np.ascontiguousarray(s)})
    return in_maps


def _deskew(yb):
    """yb: [6, 8, 16, 160, 2, 24] fp32 (one batch) -> [81, 192, 192] fp32.

    y[sp, r, p'', c'', half, t] where pixel (dh=2r+r2, dw), p'' = 8*r2+dw, and
    c'' = 16*(r2+di) + dw + dj for displacement (di, dj).
    """
    s_sp, s_r, s_p, s_c, s_half, s_t = yb.strides
    v = np.lib.stride_tricks.as_strided(
        yb,
        shape=(D, D, 2, NSP, 8, 2, N_WT, TW),
        strides=(16 * s_c, s_c, s_half, s_sp, s_r,
                 8 * s_p + 16 * s_c, s_t, s_p + s_c),
    )
    # axes: di, dj, half, sp, r, r2, t, dw -> [81, H, W]
    return np.ascontiguousarray(v).reshape(D * D, H, W)


def kernel(x1, x2):
    _install_axon_trace_shim()
    nc = _get_nc()
    in_maps = _prep_inputs(x1, x2)
    res = run_bass_kernel_spmd(nc, in_maps, core_ids=list(range(B)))
    kernel.last_results = res
    out = np.empty((B, D * D, H, W), dtype=np.float32)
    for b in range(B):
        yb = np.asarray(res.results[b]["y"]).astype(np.float32)
        out[b] = _deskew(yb)
    return out
